# revision 56
# baseline (speedup 1.0000x reference)
"""MCR2 variational loss on 8 Trainium2 NeuronCores.

Strategy (data-parallel over the sample axis n):
  - The heavy part of the loss is the per-class second-moment matrices
    M_j = Z^T diag(Pi_j) Z (plus the global gram Z^T Z), which reads all of
    Z/Pi once -> memory-bound. Everything downstream (logdet, log1p terms,
    Frobenius distance) is O(C*d^2) scalar work done on the host in fp64.
  - Fast path (Pi exactly one-hot): each sample contributes to exactly one
    class, so per-class partial grams over class-sorted rows give all M_j,
    and gram = sum_j M_j. Host distributes rows so every core gets an
    almost equal share of each class, pads each class segment to a 128-row
    multiple, and the device accumulates each class's Gram in PSUM.
  - Z ships as fp8 e4m3 (quarter of fp32 HBM traffic; measured effect on
    the final losses is ~1.5e-3 relative, an order of magnitude under the
    tolerance). The PE consumes row-chunks two at a time with the fp8
    DoubleRow perf mode (~127ns per 256 rows); odd segment tails use a
    single plain fp8 matmul (~107ns). The cadence is Tensor-sequencer
    dispatch-bound, not array-bound.
  - DMA: each per-core stream is cut into blocks pre-tiled in DRAM so
    every SBUF partition's data is one contiguous descriptor, and every
    block is split across the two HWDGE rings by partition halves. A ring
    retires one descriptor per ~14ns (or desc_bytes/145 B/ns if larger),
    so per-block flight is ~0.9us and the two rings aggregate ~290 B/ns,
    just ahead of the PE's ~258 B/ns — the plan uses a small first block
    (fast PE start), large middle blocks, and a small final pair (the last
    block's flight gates the final class). SWDGE loads measured 3-6x
    slower and regress badly; gpsimd only carries mid-stream stores.
  - PSUM: one tile per 2KB bank (classes 0-3 / 4-7 / 8 / 9). Per-bank
    tiles matter: with one big accumulator tile the Tile framework
    serializes every matmul after a drain copy behind it (write-after-read
    on the tile), stalling the PE ~1.4us per drain.
  - Output: partial M drains from PSUM as bf16 (cast on the DVE copy) per
    completed bank; mid-stream stores ride SWDGE so they never steal
    load-ring bandwidth; classes 8+9 ship as one final store split across
    both HW rings (its flight is descriptor-count-bound, so one split
    store beats two). Host all-reduces the 8 partials in fp64.
  - Fallback (general dense Pi): host BLAS contraction.
"""

import numpy as np

EPS = 0.5
MU = 1.0
C = 10
N_TOTAL = 131072
D = 128
N_CORES = 8
CHUNK = 128  # rows per PE k-tile (contraction dim)

_compiled_cache = {}

def _plan(seg_chunks):
    """PE op list + DMA block schedule for a class-sorted chunk stream.

    Returns a list of blocks (ops, n_chunks) where each op is a dict with
    cls / w (1 or 2 chunks) / q (chunk offset inside the block) / start /
    stop. Pairs (w=2) never straddle a block boundary so each DoubleRow
    matmul reads one tile.

    Every block is DMAed split across the two HWDGE rings by partition
    halves (SP/sync gets SBUF partitions 0-63, ACT/scalar 64-127). A ring
    processes one descriptor (= one SBUF partition row) every ~14 ns OR
    descriptor_bytes/145 ns, whichever is larger — with 12-18 chunk
    blocks (1.5-2.25KB descriptors) each ring spends ~0.9us per 64-row
    half, the rings stay byte-balanced by construction, and block flight
    latency is half that of whole-block-per-ring scheduling. ~8 blocks
    also keeps DMAs per engine inside the semaphore pool (reuse
    serializes issue).
    """
    ops = []
    for j, k in enumerate(seg_chunks):
        n2, n1 = divmod(k, 2)
        widths = [2] * n2 + ([1] if n1 else [])
        for i, w in enumerate(widths):
            ops.append(
                {"cls": j, "w": w, "start": i == 0, "stop": i == len(widths) - 1}
            )
    total = sum(seg_chunks)

    # every block >= 16 chunks so no flight time is wasted under the
    # 64-descriptor-per-ring floor (sum of flights then equals the
    # bandwidth minimum, bytes / 290 B/ns); a 16-chunk first block costs
    # the PE start only ~0.1us over a smaller one
    first, mid = 16, 19
    n_mid = max(1, -(-(total - first) // mid)) if total > first else 0
    targets = [first]
    if n_mid:
        base, r = divmod(total - first, n_mid)
        targets += [base + (1 if i < r else 0) for i in range(n_mid)]
    targets = [t for t in targets if t > 0]

    # pack ops to the targets, closing each block once it reaches its
    # target (ops are 1-2 chunks so overshoot is at most 1); the last
    # block absorbs any residue
    blocks = []
    ti = 0
    cur, cur_chunks = [], 0
    for op in ops:
        op = dict(op)
        op["q"] = cur_chunks
        cur.append(op)
        cur_chunks += op["w"]
        if ti < len(targets) - 1 and cur_chunks >= targets[ti]:
            blocks.append((cur, cur_chunks))
            ti += 1
            cur, cur_chunks = [], 0
    if cur:
        blocks.append((cur, cur_chunks))

    return blocks


def _build_bass_program(seg_chunks):
    """SPMD bass program computing per-class partial grams in fp8.

    Device input "z": class-sorted, zero-padded, PRE-TILED fp8 Z — for each
    DMA block of kb chunks a contiguous [128, kb*128] slab (each SBUF
    partition's data contiguous in DRAM). Output "m_out": [128, C*128]
    bf16 partial M (d on partitions, (j,e) on free).
    """
    import concourse.bacc as bacc
    import concourse.tile as tile
    from concourse import mybir
    from contextlib import ExitStack

    blocks = _plan(seg_chunks)
    total_chunks = sum(kb for _, kb in blocks)

    # Each PSUM bank is its OWN tile: classes 0-3 bank0, 4-7 bank1,
    # 8 bank2, 9 bank3. Draining a finished bank must not create a
    # write-after-read hazard on the banks the PE is still accumulating
    # into — with one big acc tile the Tile framework serializes every
    # subsequent matmul behind the drain copy (~1.4us stall per drain).
    # bank index, column offset within bank (in classes)
    psum_bank = {j: (0, j) for j in range(4)}
    psum_bank.update({j: (1, j - 4) for j in range(4, 8)})
    psum_bank[8] = (2, 0)
    psum_bank[9] = (3, 0)
    # store units: class range [a, b) stored when class b-1's cast is done.
    # A ring store costs 64 descriptors per ring regardless of width, so
    # classes 8+9 ship as ONE split store; banks 0-3 and 4-7 ship as soon
    # as they are cast, hidden behind the remaining matmul stream.
    stores = {3: (0, 4), 7: (4, 8), 9: (8, 10)}

    nc = bacc.Bacc("TRN2", target_bir_lowering=False, debug=False, num_devices=N_CORES)
    z = nc.dram_tensor(
        "z", [total_chunks * CHUNK, D], mybir.dt.float8e4, kind="ExternalInput"
    ).ap()
    out = nc.dram_tensor(
        "m_out", [D, C * D], mybir.dt.bfloat16, kind="ExternalOutput"
    ).ap()


    with tile.TileContext(nc) as tc:
        with ExitStack() as ctx:
            psum = ctx.enter_context(tc.tile_pool(name="psum", bufs=1, space="PSUM"))
            opool = ctx.enter_context(tc.tile_pool(name="o", bufs=1))
            banks = [
                psum.tile([128, 4 * D], mybir.dt.float32, name=f"bank{i}")
                for i in range(4)
            ]
            sb_out = opool.tile([128, C * D], mybir.dt.bfloat16)
            row0 = 0
            for b, (ops, kb) in enumerate(blocks):
                pool = ctx.enter_context(tc.tile_pool(name=f"z{b}", bufs=1))
                tl = pool.tile([128, kb * D], mybir.dt.float8e4)
                src = z[row0 : row0 + CHUNK * kb, :].rearrange(
                    "(p k) d -> p (k d)", p=128
                )
                nc.sync.dma_start(tl[0:64, :], src[0:64, :])
                nc.scalar.dma_start(tl[64:128, :], src[64:128, :])
                row0 += CHUNK * kb
                for op in ops:
                    j, q = op["cls"], op["q"]
                    bk, col = psum_bank[j]
                    dst = banks[bk][:, col * D : (col + 1) * D]
                    if op["w"] == 2:
                        opnd = tl[:, q * D : (q + 2) * D].rearrange(
                            "p (two d) -> p two d", two=2
                        )
                        nc.tensor.matmul(
                            dst,
                            opnd,
                            opnd,
                            start=op["start"],
                            stop=op["stop"],
                            perf_mode=mybir.MatmulPerfMode.DoubleRow,
                            skip_group_check=True,
                        )
                    else:
                        opnd = tl[:, q * D : (q + 1) * D]
                        nc.tensor.matmul(
                            dst,
                            opnd,
                            opnd,
                            start=op["start"],
                            stop=op["stop"],
                            skip_group_check=True,
                        )
                    # cast a PSUM bank out only when the whole bank is done
                    # — casting a single class early would create a write-
                    # after-read hazard with the PE still accumulating the
                    # bank's other classes; mid-stream stores ride SWDGE so
                    # they never steal load-ring bandwidth
                    if op["stop"]:
                        if j in (3, 7, 8, 9):
                            a0 = {3: 0, 7: 4, 8: 8, 9: 9}[j]
                            bk, col = psum_bank[a0]
                            csl = slice(a0 * D, (j + 1) * D)
                            nc.vector.tensor_copy(
                                sb_out[:, csl],
                                banks[bk][:, col * D : (col + (j + 1 - a0)) * D],
                            )
                        if j in stores:
                            a, bcls = stores[j]
                            sl = slice(a * D, bcls * D)
                            # all stores ride the HW rings, split by
                            # partition ranges (flight is descriptor-count
                            # bound). Ring FIFO order keeps them behind
                            # any in-flight input descriptors, so the
                            # early store costs the loads nothing — and
                            # with no SWDGE traffic at all, GpSimd's
                            # ~1.8us queue drain leaves the teardown path.
                            nc.sync.dma_start(out[0:64, sl], sb_out[0:64, sl])
                            nc.scalar.dma_start(
                                out[64:128, sl], sb_out[64:128, sl]
                            )
    nc.compile()
    return nc


def _is_one_hot(Pi):
    if not (Pi.sum(axis=1) == 1.0).all():
        return False
    if not (Pi.max(axis=1) == 1.0).all():
        return False
    return np.count_nonzero(Pi) == Pi.shape[0]


def _fast_path_M(Z, Pi):
    """Per-class second moments via the device. Returns M [C, D, D] fp64."""
    from concourse.bass_utils import run_bass_kernel_spmd
    from concourse import mybir

    fp8 = mybir.dt.np(mybir.dt.float8e4)
    labels = np.argmax(Pi, axis=1)

    # balance every class across cores: class j's rows are dealt out in
    # near-equal contiguous slices, so per-class per-core counts differ by
    # at most 1 and padding is minimal
    order = np.argsort(labels, kind="stable")
    cls_counts = np.bincount(labels, minlength=C)
    cls_offs = np.concatenate([[0], np.cumsum(cls_counts)])

    counts = np.zeros((N_CORES, C), dtype=np.int64)
    for j in range(C):
        base, rem = divmod(int(cls_counts[j]), N_CORES)
        for c in range(N_CORES):
            counts[c, j] = base + (1 if c < rem else 0)

    seg_chunks = [max(1, int(np.ceil(counts[:, j].max() / CHUNK))) for j in range(C)]

    key = tuple(seg_chunks)
    if key not in _compiled_cache:
        _compiled_cache[key] = _build_bass_program(seg_chunks)
    nc = _compiled_cache[key]

    blocks = _plan(seg_chunks)
    block_sizes = [kb for _, kb in blocks]
    total_chunks = sum(block_sizes)
    offs = np.concatenate([[0], np.cumsum(seg_chunks)]) * CHUNK

    Zq = np.ascontiguousarray(Z, dtype=np.float32).astype(fp8)
    in_maps = []
    for c in range(N_CORES):
        zbuf = np.zeros((total_chunks * CHUNK, D), dtype=fp8)
        for j in range(C):
            lo = cls_offs[j] + counts[:c, j].sum()
            nj = counts[c, j]
            zbuf[offs[j] : offs[j] + nj] = Zq[order[lo : lo + nj]]
        # pre-tile each DMA block: [kb, 128, D] -> [128, kb*D]
        parts = []
        start = 0
        for kb in block_sizes:
            blk = zbuf[start * CHUNK : (start + kb) * CHUNK]
            parts.append(
                np.ascontiguousarray(
                    blk.reshape(kb, CHUNK, D).transpose(1, 0, 2)
                ).reshape(-1)
            )
            start += kb
        zdev = np.concatenate(parts).reshape(total_chunks * CHUNK, D)
        in_maps.append({"z": zdev})

    res = run_bass_kernel_spmd(nc, in_maps, list(range(N_CORES)))
    M = np.zeros((C, D, D), dtype=np.float64)
    for c in range(N_CORES):
        o = res.results[c]["m_out"].astype(np.float64)  # [D, C*D]
        M += o.reshape(D, C, D).transpose(1, 0, 2)
    return M


def _dense_path_M(Z, Pi):
    """General dense Pi: host BLAS contraction. Returns (M, gram) fp64."""
    Zf = np.ascontiguousarray(Z, dtype=np.float32)
    A = (Pi[:, :, None].astype(np.float32) * Zf[:, None, :]).reshape(Zf.shape[0], -1)
    M = (A.T @ Zf).reshape(C, D, D).astype(np.float64)
    gram = (Zf.T @ Zf).astype(np.float64)
    return M, gram


def kernel(Z, Pi, Us):
    Z = np.asarray(Z, dtype=np.float32)
    Pi = np.asarray(Pi, dtype=np.float32)
    Us = np.asarray(Us, dtype=np.float32)
    n, d = Z.shape

    if n == N_TOTAL and d == D and Pi.shape == (n, C) and _is_one_hot(Pi):
        M = _fast_path_M(Z, Pi)
        gram = M.sum(axis=0)
    else:
        M, gram = _dense_path_M(Z, Pi)

    nf = float(n)
    df = float(d)

    A = np.eye(d, dtype=np.float64) + (df / (nf * EPS)) * gram
    sign, logabsdet = np.linalg.slogdet(A)
    loss_R = 0.5 * logabsdet

    trPi = Pi.astype(np.float64).sum(axis=0)
    col_norms_sq = (Us.astype(np.float64) ** 2).sum(axis=1)  # [C, d]
    with np.errstate(divide="ignore"):
        per_class = np.log1p((df / (trPi[:, None] * EPS)) * col_norms_sq).sum(axis=1)
    loss_Rc = ((trPi / (2.0 * nf)) * per_class).sum()

    Us64 = Us.astype(np.float64)
    UUt = np.einsum("jdk,jek->jde", Us64, Us64)
    loss_reg = 0.5 * MU * ((M - UUt) ** 2).sum()

    loss_obj = loss_R - loss_Rc - loss_reg
    return (
        np.float32(-loss_obj),
        np.float32(loss_R),
        np.float32(loss_Rc),
        np.float32(loss_reg),
    )

